# revision 14
# baseline (speedup 1.0000x reference)
"""JukeboxAttention Trainium2 kernel (bf16).

Shards the B*BLOCKS=32 independent attention blocks across 8 NeuronCores
(4 blocks = 2048 tokens per core); weights and x are pre-tiled/transposed
on the host for contiguous DMA. Per core and block:
  per-head q/k/v in [hd, tok] layout straight from x^T, causal block
  attention in [k, q] layout with triangular tile skipping, ctx in
  [q, hd] with softmax normalization fused into the PSUM->SBUF copy
  (per-partition Act scale), then out = ctx @ c_proj_w + b in f32.
"""

import sys

sys.path.insert(0, "/opt/trn_rl_repo")

import numpy as np

B, L, E = 2, 8192, 2048
HEADS, HD = 16, 128
BLOCKS, BC = 16, 512
SCALE2 = float(HD) ** -0.5  # (hd^-0.25)^2 applied to q side
NCORES = 8
BPC = B * BLOCKS // NCORES  # blocks per core = 4
T = BPC * BC  # tokens per core = 2048
ET = E // 128  # 16 contraction tiles


def _build_nc(reps=1, loop=1):
    import concourse.bass as bass  # noqa: F401
    from concourse import bacc, mybir, tile

    f32 = mybir.dt.float32
    bf16 = mybir.dt.bfloat16
    Act = mybir.ActivationFunctionType

    nc = bacc.Bacc("TRN2", target_bir_lowering=False, debug=False)

    # xst[et, p, t] = x[t, et*128+p]  (x^T, tiled over contraction dim)
    xst = nc.dram_tensor("xst", [ET, 128, T], bf16, kind="ExternalInput").ap()
    # waq_t[ft, p, et*128+j] = c_attn_w[et*128+p, ft*128+j]; ft: q=h, k=16+h, v=32+h
    waq = nc.dram_tensor("waq", [3 * ET, 128, E], bf16, kind="ExternalInput").ap()
    cab = nc.dram_tensor("cab", [128, 3 * ET], f32, kind="ExternalInput").ap()
    # wp_t[f, dg, p, dtl*512+j] = c_proj_w[dg*512+dtl*128+p, f*512+j]
    wp = nc.dram_tensor("wp", [4, 4, 128, E], bf16, kind="ExternalInput").ap()
    cpb = nc.dram_tensor("cpb", [E], f32, kind="ExternalInput").ap()
    maskt = nc.dram_tensor("maskt", [128, 128], bf16, kind="ExternalInput").ap()
    ident = nc.dram_tensor("ident", [128, 128], bf16, kind="ExternalInput").ap()
    out = nc.dram_tensor("out", [T, E], f32, kind="ExternalOutput").ap()

    def emit_block(pools, consts, blk):
        (xtp, wload, ctxp, qkvp, vhp, prp, csp, rcp, outp,
         psA, psC, psD, psT) = pools
        (ident_sb, mask_sb, cab_sb, pbias_bc, ones_col) = consts
        t0 = blk * BC

        # ---- x^T tiles straight from DRAM ----
        Xt = xtp.tile([128, ET, BC], bf16, tag="xt")
        nc.sync.dma_start(
            out=Xt, in_=xst.rearrange("et p t -> p et t")[:, :, t0:t0 + BC])

        # ---- per-head qkv + causal attention ----
        ctxT = ctxp.tile([128, HEADS, BC], bf16, tag="ctxt")
        for h in range(HEADS):
            wq = wload.tile([128, ET, 128], bf16, tag="w")
            nc.sync.dma_start(out=wq, in_=waq[h].rearrange("p (et j) -> p et j", et=ET))
            wk = wload.tile([128, ET, 128], bf16, tag="w")
            nc.gpsimd.dma_start(out=wk, in_=waq[ET + h].rearrange("p (et j) -> p et j", et=ET))
            wv = wload.tile([128, ET, 128], bf16, tag="w")
            nc.sync.dma_start(out=wv, in_=waq[2 * ET + h].rearrange("p (et j) -> p et j", et=ET))

            # q, k, v in [hd, tok]
            ps_q = psA.tile([128, BC], f32, tag="psa")
            for et in range(ET):
                nc.tensor.matmul(ps_q, lhsT=wq[:, et, :], rhs=Xt[:, et, :],
                                 start=(et == 0), stop=(et == ET - 1))
            q_sb = qkvp.tile([128, BC], bf16, tag="qkv")
            nc.scalar.activation(q_sb, ps_q, Act.Identity,
                                 bias=cab_sb[:, h:h + 1], scale=SCALE2)

            ps_k = psA.tile([128, BC], f32, tag="psa")
            for et in range(ET):
                nc.tensor.matmul(ps_k, lhsT=wk[:, et, :], rhs=Xt[:, et, :],
                                 start=(et == 0), stop=(et == ET - 1))
            k_sb = qkvp.tile([128, BC], bf16, tag="qkv")
            nc.scalar.activation(k_sb, ps_k, Act.Identity,
                                 bias=cab_sb[:, ET + h:ET + h + 1], scale=1.0)

            ps_v = psA.tile([128, BC], f32, tag="psa")
            for et in range(ET):
                nc.tensor.matmul(ps_v, lhsT=wv[:, et, :], rhs=Xt[:, et, :],
                                 start=(et == 0), stop=(et == ET - 1))
            v_sb = qkvp.tile([128, BC], bf16, tag="qkv")
            nc.scalar.activation(v_sb, ps_v, Act.Identity,
                                 bias=cab_sb[:, 2 * ET + h:2 * ET + h + 1], scale=1.0)

            # v^T -> v_head [k, (kt, hd)] for ctx rhs
            ps_vt = psT.tile([128, 4 * 128], bf16, tag="pst")
            for kt in range(4):
                nc.tensor.transpose(ps_vt[:, kt * 128:(kt + 1) * 128],
                                    v_sb[:, kt * 128:(kt + 1) * 128], ident_sb)
            v_head = vhp.tile([128, 4, 128], bf16, tag="vh")
            nc.vector.tensor_copy(v_head, ps_vt)

            # scores^T [k, q] -> exp -> diagonal mask; triangular tile skip
            pbs = []
            for kt in range(4):
                qlen = BC - kt * 128
                ps_s = psA.tile([128, BC], f32, tag="psa")
                nc.tensor.matmul(ps_s[:, :qlen],
                                 lhsT=k_sb[:, kt * 128:(kt + 1) * 128],
                                 rhs=q_sb[:, kt * 128:], start=True, stop=True)
                pb = prp.tile([128, BC], bf16, tag="pb")
                nc.scalar.activation(pb[:, :qlen], ps_s[:, :qlen], Act.Exp)
                nc.vector.tensor_mul(pb[:, :128], pb[:, :128], mask_sb)
                pbs.append(pb)

            # ctx [q, (qt, hd)] and denominators ps_d[:, qt] = sum_k probs;
            # the denom matmul shares its lhsT (probs slice) with the ctx
            # matmul just before it, so the PE stationary load is reused
            ps_d = psD.tile([128, 4], f32, tag="psd")
            ps_c = psC.tile([128, BC], f32, tag="psc")
            for qt in range(4):
                for kt in range(qt + 1):
                    off = (qt - kt) * 128
                    nc.tensor.matmul(ps_c[:, qt * 128:(qt + 1) * 128],
                                     lhsT=pbs[kt][:, off:off + 128],
                                     rhs=v_head[:, kt, :],
                                     start=(kt == 0), stop=(kt == qt))
                    nc.tensor.matmul(ps_d[:, qt:qt + 1],
                                     lhsT=pbs[kt][:, off:off + 128], rhs=ones_col,
                                     start=(kt == 0), stop=(kt == qt))
            recip = rcp.tile([128, 4], f32, tag="recip")
            nc.vector.reciprocal(recip, ps_d)

            ctx_sb = csp.tile([128, 4, 128], bf16, tag="csb")
            for qt in range(4):
                nc.scalar.activation(ctx_sb[:, qt, :],
                                     ps_c[:, qt * 128:(qt + 1) * 128], Act.Identity,
                                     scale=recip[:, qt:qt + 1])

            # transpose ctx -> ctxT[hd, q]
            ps_t = psT.tile([128, 4 * 128], bf16, tag="pst")
            for qt in range(4):
                nc.tensor.transpose(ps_t[:, qt * 128:(qt + 1) * 128],
                                    ctx_sb[:, qt, :], ident_sb)
            if h % 2 == 0:
                nc.vector.tensor_copy(ctxT[:, h, :], ps_t)
            else:
                nc.scalar.copy(ctxT[:, h, :], ps_t)

        # ---- out = ctx @ c_proj_w + b (m-wave pipelined on 2 PSUM banks) ----
        for f in range(4):
            wpgs = []
            for dg in range(4):
                wpg = wload.tile([128, 4, BC], bf16, tag="w")
                eng = nc.sync if dg % 2 == 0 else nc.gpsimd
                eng.dma_start(out=wpg, in_=wp[f, dg].rearrange("p (dt j) -> p dt j", dt=4))
                wpgs.append(wpg)
            for m in range(4):
                ps_o = psC.tile([128, BC], f32, tag="psc")
                for dg in range(4):
                    for dtl in range(4):
                        dt = dg * 4 + dtl
                        nc.tensor.matmul(
                            ps_o,
                            lhsT=ctxT[:, dt, m * 128:(m + 1) * 128],
                            rhs=wpgs[dg][:, dtl, :],
                            start=(dt == 0), stop=(dt == 15),
                        )
                osb = outp.tile([128, BC], f32, tag="osb")
                nc.vector.tensor_add(osb, ps_o, pbias_bc[:, f * 512:(f + 1) * 512])
                nc.gpsimd.dma_start(
                    out=out[t0 + m * 128: t0 + (m + 1) * 128, f * 512:(f + 1) * 512],
                    in_=osb,
                )

    with tile.TileContext(nc) as tc:
        with (
            tc.tile_pool(name="const", bufs=1) as const,
            tc.tile_pool(name="wload", bufs=12) as wload,
            tc.tile_pool(name="xt", bufs=3) as xtp,
            tc.tile_pool(name="ctxt", bufs=2) as ctxp,
            tc.tile_pool(name="qkv", bufs=7) as qkvp,
            tc.tile_pool(name="vh", bufs=2) as vhp,
            tc.tile_pool(name="probs", bufs=8) as prp,
            tc.tile_pool(name="csb", bufs=2) as csp,
            tc.tile_pool(name="recip", bufs=2) as rcp,
            tc.tile_pool(name="outp", bufs=3) as outp,
            tc.tile_pool(name="psA", bufs=4, space="PSUM") as psA,
            tc.tile_pool(name="psC", bufs=2, space="PSUM") as psC,
            tc.tile_pool(name="psD", bufs=1, space="PSUM") as psD,
            tc.tile_pool(name="psT", bufs=1, space="PSUM") as psT,  # 4+2+1+1 = 8 banks
        ):
            # ---- constants ----
            ident_sb = const.tile([128, 128], bf16, tag="ident")
            nc.sync.dma_start(out=ident_sb, in_=ident)
            mask_sb = const.tile([128, 128], bf16, tag="mask")
            nc.sync.dma_start(out=mask_sb, in_=maskt)
            cab_sb = const.tile([128, 3 * ET], f32, tag="cab")
            nc.sync.dma_start(out=cab_sb, in_=cab)
            pbias_bc = const.tile([128, E], f32, tag="pbias")
            pb_ap = bass.AP(tensor=cpb.tensor, offset=cpb.offset,
                            ap=[[0, 128], [1, E]])
            nc.gpsimd.dma_start(out=pbias_bc, in_=pb_ap)
            ones_col_b = const.tile([128, 1], bf16, tag="ones_col")
            nc.vector.memset(ones_col_b, 1.0)

            pools = (xtp, wload, ctxp, qkvp, vhp, prp, csp, rcp, outp,
                     psA, psC, psD, psT)
            consts = (ident_sb, mask_sb, cab_sb, pbias_bc, ones_col_b)

            if loop > 1:
                with tc.For_i(0, loop):
                    for blk_i in range(BPC * reps):
                        emit_block(pools, consts, blk_i % BPC)
            else:
                for blk_i in range(BPC * reps):
                    emit_block(pools, consts, blk_i % BPC)
    nc.compile()
    return nc


_NC = {}


def _get_nc(reps=1, loop=1):
    key = (reps, loop)
    if key not in _NC:
        _NC[key] = _build_nc(reps, loop)
    return _NC[key]


def make_in_maps(x, c_attn_w, c_attn_b, c_proj_w, c_proj_b):
    import ml_dtypes

    bf = ml_dtypes.bfloat16
    x = np.asarray(x, np.float32)
    c_attn_w = np.asarray(c_attn_w, np.float32)
    c_proj_w = np.asarray(c_proj_w, np.float32)
    c_attn_b = np.asarray(c_attn_b, np.float32)
    c_proj_b = np.asarray(c_proj_b, np.float32)

    # waq_t[ft, p, et, j] = c_attn_w[et*128+p, ft*128+j]
    waq_t = np.ascontiguousarray(
        c_attn_w.reshape(ET, 128, 3 * ET, 128).transpose(2, 1, 0, 3)
        .reshape(3 * ET, 128, E).astype(bf))
    # wp_t[f, dg, p, dtl, j] = c_proj_w[dg*512+dtl*128+p, f*512+j]
    wp_t = np.ascontiguousarray(
        c_proj_w.reshape(4, 4, 128, 4, 512).transpose(3, 0, 2, 1, 4)
        .reshape(4, 4, 128, E).astype(bf))
    # q/k/v biases, q side pre-scaled (activation: out = scale*in + bias)
    b_mod = c_attn_b.copy()
    b_mod[:E] *= SCALE2
    cab = np.ascontiguousarray(b_mod.reshape(3 * ET, 128).T)
    # within-diagonal-tile causal mask: col(query) >= row(key)
    p = np.arange(128)[:, None]
    c = np.arange(128)[None, :]
    maskt = np.ascontiguousarray((c >= p).astype(bf))
    ident = np.eye(128, dtype=bf)

    xr = x.reshape(B * BLOCKS, BC, E)
    in_maps = []
    for core in range(NCORES):
        xs = xr[core * BPC:(core + 1) * BPC].reshape(T, E)
        # xst[et, p, t] = xs[t, et*128+p]
        xst = np.ascontiguousarray(
            xs.T.reshape(ET, 128, T).astype(bf))
        in_maps.append({
            "xst": xst, "waq": waq_t, "cab": cab, "wp": wp_t,
            "cpb": c_proj_b, "maskt": maskt, "ident": ident,
        })
    return in_maps


def kernel(x, c_attn_w, c_attn_b, c_proj_w, c_proj_b):
    from concourse import bass_utils

    nc = _get_nc()
    in_maps = make_in_maps(x, c_attn_w, c_attn_b, c_proj_w, c_proj_b)
    res = bass_utils.run_bass_kernel_spmd(nc, in_maps, core_ids=list(range(NCORES)))
    outs = [res.results[c]["out"] for c in range(NCORES)]
    full = np.concatenate(outs, axis=0).reshape(B, L, E).astype(np.float32)
    return full
